# revision 30
# baseline (speedup 1.0000x reference)
"""Trainium2 Bass kernel for nn_CliquesOutputLayer (self-contained).

kernel(**inputs) -> np.ndarray [4, 160000] float32.

Sharding: one formula per NeuronCore (F = 8 = n_cores).

Gather: InstDMAGatherAnt (`dma_gather`, mlp ucode library, non-transposed,
multi-packet). Q7 descriptor generation runs on one core-pair per
queue_num, so gathers round-robin queue_num 0..3 to use all 4 Q7
core-pairs concurrently (num_swdge_queues=4). Each gather fetches
NI=3072 rows = 2 compute chunks, amortizing the ~10us/pair fixed cost.
(The transposed-gather mode would skip the PE transposes below, but
concurrent transposed gathers corrupt each other through the shared
xbar, and serializing them leaves 4x descriptor-gen throughput on the
table -- non-transpose + PE transposes is strictly faster.)

dma_gather needs int16 indices, so the host compacts the atoms table per
10-chunk segment (<= 15360 draws -> unique rows fit a 16384-row table and
local ids fit int16 deterministically). Table rows pack [b0|b1|b2|b3] x 64
bf16 (512B) so one gathered row serves all 4 batches.

Compute per 512-grounding chunk: 24 PE transposes ([128 g, 128 feat] bf16
-> bf16 PSUM, batch-pair-packed), 3 PSUM->SBUF copies (DVE x2, ACT x1),
layer 1 as 6 accumulating bf16 matmuls (batch pairs (0,1)/(2,3) share
K=128 via block-diagonal W1), one [64,512] sigmoid, layer 2 as one K=64
block-diagonal matmul, one [4,512] sigmoid, store. PE/ACT software
pipelines are skewed so no engine round-trips stall PE.
"""
from contextlib import ExitStack
from dataclasses import dataclass
import numpy as np
import concourse.bass as bass
import concourse.mybir as mybir

F32 = mybir.dt.float32
BF16 = mybir.dt.bfloat16
I16 = mybir.dt.int16


@dataclass
class Cfg:
    n_atoms: int = 100000
    g: int = 20000
    b: int = 4
    d: int = 64
    h: int = 16
    chunks_per_gather: int = 4
    gat_bufs: int = 4
    seg_cap: int = 16384

    @property
    def gpad(self):
        blocks = (self.g + 127) // 128
        align = 4 * self.chunks_per_gather
        return (blocks + align - 1) // align * align * 128

    @property
    def nchunk(self):
        return self.gpad // 512

    @property
    def ngather(self):
        return len(self.gather_plan)

    @property
    def gather_plan(self):
        # Segment-aware plan: each compaction segment (chunks_per_seg chunks)
        # packs floor(CPS/CPG) big gathers plus 1-chunk smalls for the
        # remainder. Segment 0 leads with its smalls (cheap opening
        # Pool-engine sync-block); later segments trail them (fine-grained
        # drain). No gather ever spans a segment boundary by construction.
        nc_ = self.nchunk
        CPS, CPG = self.chunks_per_seg, self.chunks_per_gather
        plan = []
        s0 = 0
        while s0 < nc_:
            span = min(CPS, nc_ - s0)
            nbig = span // CPG
            rem = span - nbig * CPG
            if s0 == 0:
                for k in range(rem):
                    plan.append((s0 + k, 1))
                for k in range(nbig):
                    plan.append((s0 + rem + k * CPG, CPG))
            else:
                for k in range(nbig):
                    plan.append((s0 + k * CPG, CPG))
                for k in range(rem):
                    plan.append((s0 + nbig * CPG + k, 1))
            s0 += span
        assert sum(n for _, n in plan) == nc_, plan
        return plan

    @property
    def chunks_per_seg(self):
        return max(1, self.seg_cap // 1536)

    @property
    def nseg(self):
        return (self.nchunk + self.chunks_per_seg - 1) // self.chunks_per_seg


def build_nc(cfg: Cfg) -> bass.Bass:
    from concourse import library_config

    B, D, H = cfg.b, cfg.d, cfg.h
    BD = B * D              # 256 elements per table row
    NC = cfg.nchunk
    PLAN = cfg.gather_plan
    NG = len(PLAN)
    CPG = cfg.chunks_per_gather
    NGB = cfg.gat_bufs
    IW1 = 1536 // 16        # idx words per chunk
    # chunk -> (gather idx, offset within gather); gather -> idx-word start
    chunk_gather = {}
    g_iw0 = []
    iw = 0
    for gi, (c0, nch) in enumerate(PLAN):
        g_iw0.append(iw)
        for k in range(nch):
            chunk_gather[c0 + k] = (gi, k)
        iw += nch * IW1
    TOT_IW = iw

    nc = bass.Bass(trn_type="TRN2", num_swdge_queues=4)
    e2c = nc.declare_dram_parameter(
        "e2c", [cfg.nseg, cfg.seg_cap, BD], BF16, isOutput=False)
    gidx = nc.declare_dram_parameter("gidx", [128, TOT_IW], I16, isOutput=False)
    w1blk = nc.declare_dram_parameter("w1blk", [128, 3, 2 * H], BF16, isOutput=False)
    w2q = nc.declare_dram_parameter("w2q", [4 * H, B], BF16, isOutput=False)
    b1q = nc.declare_dram_parameter("b1q", [4 * H, 1], F32, isOutput=False)
    b2q = nc.declare_dram_parameter("b2q", [B, 1], F32, isOutput=False)
    iden = nc.declare_dram_parameter("iden", [128, 128], BF16, isOutput=False)
    o2d = nc.declare_dram_parameter("o2d", [B, cfg.gpad], F32, isOutput=True)

    with ExitStack() as ctx:
        # gathered rows: [g-in-block 128, buf, block (CPG*12), row 256]
        gat = ctx.enter_context(
            nc.sbuf_tensor("gat", [128, NGB, CPG * 12, BD], BF16))
        gidx_sb = ctx.enter_context(nc.sbuf_tensor("gidx_sb", [128, TOT_IW], I16))
        dscr = ctx.enter_context(nc.sbuf_tensor("dscr", [128, 4, BD], BF16))
        w1_sb = ctx.enter_context(nc.sbuf_tensor("w1_sb", [128, 3, 2 * H], BF16))
        w2_sb = ctx.enter_context(nc.sbuf_tensor("w2_sb", [4 * H, B], BF16))
        b1_sb = ctx.enter_context(nc.sbuf_tensor("b1_sb", [4 * H, 1], F32))
        b2_sb = ctx.enter_context(nc.sbuf_tensor("b2_sb", [B, 1], F32))
        id_sb = ctx.enter_context(nc.sbuf_tensor("id_sb", [128, 128], BF16))
        # post-transpose activations: [feat, pair01 512 | pair23 512] x2 bufs
        xsb = [
            ctx.enter_context(nc.sbuf_tensor(f"xsb{sl}", [128, 2, 1024], BF16))
            for sl in range(3)
        ]
        h_sb = ctx.enter_context(nc.sbuf_tensor("h_sb", [4 * H, 2, 512], BF16))
        o_sb = ctx.enter_context(nc.sbuf_tensor("o_sb", [B, 2, 512], F32))
        # PSUM: 3 transpose slabs (bf16, 1 bank per buf) + h/o slab (f32)
        pt = [
            ctx.enter_context(nc.psum_tensor(f"pt{sl}", [128, 2, 1024], BF16))
            for sl in range(3)
        ]
        pho = ctx.enter_context(nc.psum_tensor("pho", [4 * H + B, 2, 512], F32))

        io = ctx.enter_context(nc.semaphore("io"))
        wio = ctx.enter_context(nc.semaphore("wio"))
        gs = [ctx.enter_context(nc.semaphore(f"gs{c}")) for c in range(8)]
        tp_s = ctx.enter_context(nc.semaphore("tp_s"))
        cpv_s = ctx.enter_context(nc.semaphore("cpv_s"))
        cpa_s = ctx.enter_context(nc.semaphore("cpa_s"))
        mm1_s = ctx.enter_context(nc.semaphore("mm1_s"))
        hs_s = ctx.enter_context(nc.semaphore("hs_s"))
        mm2_s = ctx.enter_context(nc.semaphore("mm2_s"))
        os_s = ctx.enter_context(nc.semaphore("os_s"))
        st_p = [ctx.enter_context(nc.semaphore(f"st_p{i}")) for i in range(2)]
        dm = [ctx.enter_context(nc.semaphore(f"dm{i}")) for i in range(4)]
        block = ctx.enter_context(nc.Block())

        @block.sync
        def _(sync):
            sync.dma_start(out=gidx_sb[:], in_=gidx[:]).then_inc(io, 16)
            sync.dma_start(out=w1_sb[:], in_=w1blk[:]).then_inc(wio, 16)
            sync.dma_start(out=w2_sb[:], in_=w2q[:]).then_inc(wio, 16)
            sync.dma_start(out=b1_sb[:], in_=b1q[:]).then_inc(wio, 16)
            sync.dma_start(out=b2_sb[:], in_=b2q[:]).then_inc(wio, 16)
            sync.dma_start(out=id_sb[:], in_=iden[:]).then_inc(wio, 16)
            for q in range(NC):
                sync.wait_ge(os_s, q + 1)
                if q >= 2:
                    sync.wait_ge(st_p[q % 2], 16 * (q // 2))
                sync.dma_start(
                    out=o2d[:, q * 512:(q + 1) * 512], in_=o_sb[:, q % 2, :]
                ).then_inc(st_p[q % 2], 16)
            for par in range(2):
                sync.wait_ge(st_p[par], 16 * ((NC + 1 - par) // 2))

        @block.gpsimd
        def _(gpsimd):
            gpsimd.load_library(library_config.mlp)
            gpsimd.wait_ge(io, 16)
            for g in range(NG):
                c0, nch = PLAN[g]
                assert c0 // cfg.chunks_per_seg == (c0 + nch - 1) // cfg.chunks_per_seg, (
                    f"gather {g} spans segment boundary: {PLAN[g]}")
                if g >= NGB:
                    # gat buffer free when its prior chunks' transposes done
                    pc0, pn = PLAN[g - NGB]
                    gpsimd.wait_ge(tp_s, 24 * (pc0 + pn))
                if g >= 8:
                    # 2 alternating sems per queue: this wait is 2 rounds
                    # back so the sequencer never stalls on it
                    gpsimd.wait_ge(gs[g % 8], 16 * (g // 8))
                gpsimd.dma_gather(
                    out_ap=gat[:, g % NGB, 0:12 * nch, :],
                    in_ap=e2c[c0 // cfg.chunks_per_seg],
                    idxs_ap=gidx_sb[:, g_iw0[g]:g_iw0[g] + nch * IW1],
                    num_idxs=1536 * nch,
                    num_idxs_reg=1536 * nch,
                    elem_size=BD,
                    transpose=False,
                    single_packet=False,
                    queue_num=(g + 1) % 4,
                ).then_inc(gs[g % 8], 16)
            for c in range(8):
                gpsimd.wait_ge(gs[c], 16 * ((NG - c + 7) // 8))

        def pe_transposes(tensor, q):
            # 24 transposes: [128 g, 128 (bpair,d)] -> bf16 PSUM
            bq = q % 2
            g, koff = chunk_gather[q]
            tensor.wait_ge(gs[g % 8], 16 * (g // 8 + 1))
            if q >= 2:
                tensor.wait_ge(cpv_s, 2 * (q - 1))
                tensor.wait_ge(cpa_s, q - 1)
            cbase = koff * 12
            for j in range(4):
                for sl in range(3):
                    for hf in range(2):
                        tensor.matmul(
                            pt[sl][:, bq, hf * 512 + j * 128:hf * 512 + (j + 1) * 128],
                            gat[:, g % NGB, cbase + j * 3 + sl,
                                hf * 128:(hf + 1) * 128],
                            id_sb[:], is_transpose=True,
                        ).then_inc(tp_s, 1)

        def pe_layer1(tensor, q):
            # layer 1: 2 pairs x 3 slots, K=128 block-diagonal W1
            bq = q % 2
            tensor.wait_ge(cpv_s, 2 * (q + 1))
            tensor.wait_ge(cpa_s, q + 1)
            if q >= 2:
                tensor.wait_ge(hs_s, q - 1)
            for p in range(2):
                for sl in range(3):
                    mm = tensor.matmul(
                        pho[p * 2 * H:(p + 1) * 2 * H, bq, :],
                        w1_sb[:, sl, :],
                        xsb[sl][:, bq, p * 512:(p + 1) * 512],
                        start=(sl == 0), stop=(sl == 2),
                    )
                    if sl == 2:
                        mm.then_inc(mm1_s, 1)

        def pe_layer2(tensor, q):
            # layer 2: K=64, 4-batch block-diagonal W2
            bq = q % 2
            tensor.wait_ge(hs_s, q + 1)
            if q >= 2:
                tensor.wait_ge(os_s, q - 1)
            tensor.matmul(
                pho[4 * H:4 * H + B, bq, :], w2_sb[:], h_sb[:, bq, :],
                start=True, stop=True,
            ).then_inc(mm2_s, 1)

        @block.tensor
        def _(tensor):
            tensor.wait_ge(wio, 16 * 5)
            # software pipeline: T[q] | L1[q-1] | L2[q-2] keeps PE fed while
            # DVE/ACT drain PSUM and ACT computes sigmoids
            for q in range(NC):
                pe_transposes(tensor, q)
                if q >= 1:
                    pe_layer1(tensor, q - 1)
                if q >= 2:
                    pe_layer2(tensor, q - 2)
            pe_layer1(tensor, NC - 1)
            pe_layer2(tensor, NC - 2)
            pe_layer2(tensor, NC - 1)
            tensor.wait_ge(mm2_s, NC)

        @block.vector
        def _(vector):
            for q in range(NC):
                bq = q % 2
                vector.wait_ge(tp_s, 24 * (q + 1))
                if q >= 2:
                    vector.wait_ge(mm1_s, 2 * (q - 1))
                vector.tensor_copy(
                    out=xsb[0][:, bq, :], in_=pt[0][:, bq, :]
                ).then_inc(cpv_s, 1)
                vector.tensor_copy(
                    out=xsb[1][:, bq, :], in_=pt[1][:, bq, :]
                ).then_inc(cpv_s, 1)
            vector.wait_ge(cpv_s, 2 * NC)

        def act_copy(scalar, q):
            bq = q % 2
            scalar.wait_ge(tp_s, 24 * (q + 1))
            if q >= 2:
                scalar.wait_ge(mm1_s, 2 * (q - 1))
            scalar.copy(
                out=xsb[2][:, bq, :], in_=pt[2][:, bq, :]
            ).then_inc(cpa_s, 1)

        def act_hsig(scalar, q):
            bq = q % 2
            scalar.wait_ge(mm1_s, 2 * (q + 1))
            scalar.activation(
                h_sb[:, bq, :], pho[0:4 * H, bq, :],
                mybir.ActivationFunctionType.Sigmoid, bias=b1_sb[:],
            ).then_inc(hs_s, 1)

        def act_osig(scalar, q):
            bq = q % 2
            scalar.wait_ge(mm2_s, q + 1)
            if q >= 2:
                scalar.wait_ge(st_p[q % 2], 16 * (q // 2))
            scalar.activation(
                o_sb[:, bq, :], pho[4 * H:4 * H + B, bq, :],
                mybir.ActivationFunctionType.Sigmoid, bias=b2_sb[:],
            ).then_inc(os_s, 1)

        @block.scalar
        def _(scalar):
            # skewed to match the PE pipeline: C[q] | Hs[q-1] | Os[q-2]
            for q in range(NC):
                act_copy(scalar, q)
                if q >= 1:
                    act_hsig(scalar, q - 1)
                if q >= 2:
                    act_osig(scalar, q - 2)
            act_hsig(scalar, NC - 1)
            act_osig(scalar, NC - 2)
            act_osig(scalar, NC - 1)
            scalar.wait_ge(os_s, NC)

    # populate .instr bytes for InstISA subclasses (the library reload);
    # without this the NEFF compiler fails with "ISA wrong length"
    from concourse.library_overlay import lower_extended_insts
    lower_extended_insts(nc)
    return nc


def host_inputs(cfg: Cfg, atoms_embeddings, grounding_indices, W1, b1, W2, b2):
    import ml_dtypes

    B, D, H = cfg.b, cfg.d, cfg.h
    NC, CPS = cfg.nchunk, cfg.chunks_per_seg
    PLAN = cfg.gather_plan
    Fn = grounding_indices.shape[0]
    e2 = np.ascontiguousarray(
        np.transpose(np.asarray(atoms_embeddings, np.float32), (1, 0, 2))
    ).reshape(cfg.n_atoms, B * D).astype(ml_dtypes.bfloat16)
    iden = np.eye(128, dtype=np.float32).astype(ml_dtypes.bfloat16)
    # wrap map: index position i -> (partition i%16 (replicated), word i//16)
    pmod = (np.arange(128) % 16)[:, None]
    maps = []
    for f in range(Fn):
        gi = np.asarray(grounding_indices[f], np.int64)
        gpadded = np.zeros((cfg.gpad, 3), np.int32)
        gpadded[:cfg.g] = gi.astype(np.int32)
        e2cs = np.zeros((cfg.nseg, cfg.seg_cap, B * D), ml_dtypes.bfloat16)
        inv_all = np.zeros((cfg.gpad, 3), np.int16)
        for s in range(cfg.nseg):
            seg = gpadded[s * CPS * 512:(s + 1) * CPS * 512]
            uniq, inv = np.unique(seg, return_inverse=True)
            assert len(uniq) <= cfg.seg_cap
            e2cs[s, :len(uniq)] = e2[uniq]
            inv_all[s * CPS * 512:(s + 1) * CPS * 512] = (
                inv.reshape(seg.shape).astype(np.int16))
        gidx_cols = []
        for c0, nch in PLAN:
            # gather row index i = c*128 + p lands at gat[p, c, :], with
            # c = chunk_in_gather*12 + j*3 + sl and p = g-in-block
            ni = 1536 * nch
            seg_inv = inv_all[c0 * 512:(c0 + nch) * 512]  # [nch*512, 3]
            arr = (seg_inv
                   .reshape(nch, 4, 128, 3)       # [cq, j, p, sl]
                   .transpose(0, 1, 3, 2)         # [cq, j, sl, p]
                   .reshape(ni))
            words = np.arange(ni // 16)[None, :]
            gidx_cols.append(arr[words * 16 + pmod])              # [128, ni/16]
        gidxf = np.concatenate(gidx_cols, axis=1)
        w1f = np.asarray(W1[f], np.float32)      # [192, 16]
        w2f = np.asarray(W2[f], np.float32)      # [16, 1]
        w1b = np.zeros((128, 3, 2 * H), np.float32)
        for sl in range(3):
            w1b[0:D, sl, 0:H] = w1f[sl * D:(sl + 1) * D]
            w1b[D:2 * D, sl, H:2 * H] = w1f[sl * D:(sl + 1) * D]
        w2b = np.zeros((4 * H, B), np.float32)
        for bp in range(B):
            w2b[bp * H:(bp + 1) * H, bp] = w2f[:, 0]
        b1v = np.asarray(b1[f], np.float32)
        maps.append({
            "e2c": e2cs,
            "gidx": gidxf,
            "w1blk": w1b.astype(ml_dtypes.bfloat16),
            "w2q": w2b.astype(ml_dtypes.bfloat16),
            "b1q": np.tile(b1v, B)[:, None].copy(),
            "b2q": np.full((B, 1), np.float32(np.asarray(b2[f]).ravel()[0])),
            "iden": iden,
        })
    return maps


def assemble(cfg: Cfg, results):
    Fn = len(results)
    out = np.zeros((cfg.b, Fn * cfg.g), np.float32)
    for f in range(Fn):
        out[:, f * cfg.g:(f + 1) * cfg.g] = results[f]["o2d"][:, :cfg.g]
    return out


_RUNTIME = {}


def _get_runtime():
    if "nc" not in _RUNTIME:
        cfg = Cfg()
        _RUNTIME["cfg"] = cfg
        _RUNTIME["nc"] = build_nc(cfg)
    return _RUNTIME["cfg"], _RUNTIME["nc"]


def kernel(atoms_embeddings, grounding_indices, W1, b1, W2, b2):
    from concourse.bass_utils import run_bass_kernel_spmd

    cfg, nc = _get_runtime()
    maps = host_inputs(cfg, atoms_embeddings, grounding_indices, W1, b1, W2, b2)
    res = run_bass_kernel_spmd(nc, maps, list(range(len(maps))))
    return assemble(cfg, [res.results[i] for i in range(len(maps))]).astype(np.float32)


# revision 32
# speedup vs baseline: 1.4144x; 1.4144x over previous
"""Trainium2 Bass kernel for nn_CliquesOutputLayer (self-contained).

kernel(**inputs) -> np.ndarray [4, 160000] float32.

Sharding: one formula per NeuronCore (F = 8 = n_cores).

Gather: InstDMAGatherAnt (`dma_gather`, mlp ucode library, non-transposed,
multi-packet). Q7 descriptor generation runs on one core-pair per
queue_num, so gathers round-robin queue_num 0..3 to use all 4 Q7
core-pairs concurrently (num_swdge_queues=4). Each gather fetches
NI=3072 rows = 2 compute chunks, amortizing the ~10us/pair fixed cost.
(The transposed-gather mode would skip the PE transposes below, but
concurrent transposed gathers corrupt each other through the shared
xbar, and serializing them leaves 4x descriptor-gen throughput on the
table -- non-transpose + PE transposes is strictly faster.)

dma_gather needs int16 indices, so the host compacts the atoms table per
10-chunk segment (<= 15360 draws -> unique rows fit a 16384-row table and
local ids fit int16 deterministically). Table rows pack [b0|b1|b2|b3] x 64
bf16 (512B) so one gathered row serves all 4 batches.

Compute per 512-grounding chunk: 24 PE transposes ([128 g, 128 feat] bf16
-> bf16 PSUM, batch-pair-packed), 3 PSUM->SBUF copies (DVE x2, ACT x1),
layer 1 as 6 accumulating bf16 matmuls (batch pairs (0,1)/(2,3) share
K=128 via block-diagonal W1), one [64,512] sigmoid, layer 2 as one K=64
block-diagonal matmul, one [4,512] sigmoid, store. PE/ACT software
pipelines are skewed so no engine round-trips stall PE.
"""
from contextlib import ExitStack
from dataclasses import dataclass
import numpy as np
import concourse.bass as bass
import concourse.mybir as mybir

F32 = mybir.dt.float32
BF16 = mybir.dt.bfloat16
I16 = mybir.dt.int16


@dataclass
class Cfg:
    n_atoms: int = 100000
    g: int = 20000
    b: int = 4
    d: int = 64
    h: int = 16
    chunks_per_gather: int = 2
    gat_bufs: int = 8
    seg_cap: int = 16384

    @property
    def gpad(self):
        blocks = (self.g + 127) // 128
        align = 4 * self.chunks_per_gather
        return (blocks + align - 1) // align * align * 128

    @property
    def nchunk(self):
        return self.gpad // 512

    @property
    def ngather(self):
        return len(self.gather_plan)

    @property
    def gather_plan(self):
        # list of (start_chunk, n_chunks); big gathers first, then 1-chunk
        # gathers for a fine-grained tail. Count kept a multiple of 4 so
        # queue rounds stay balanced.
        nc_ = self.nchunk
        # head smalls halve the opening Pool-engine sync-block; head must be
        # EVEN so big gathers start at even chunks and never span a
        # compaction-segment boundary (segments are 10 chunks)
        head = 4 if nc_ >= 16 else 0
        tail = 4 if nc_ >= 16 else (nc_ if nc_ < 12 else 4)
        big = nc_ - head - tail
        if big % self.chunks_per_gather:
            tail += big % self.chunks_per_gather
            big -= big % self.chunks_per_gather
        plan = []
        c = 0
        for _ in range(head):
            plan.append((c, 1)); c += 1
        for _ in range(big // self.chunks_per_gather):
            plan.append((c, self.chunks_per_gather)); c += self.chunks_per_gather
        for _ in range(tail):
            plan.append((c, 1)); c += 1
        assert c == nc_ and len(plan) % 4 == 0, (c, nc_, plan)
        return plan

    @property
    def chunks_per_seg(self):
        return max(1, self.seg_cap // 1536)

    @property
    def nseg(self):
        return (self.nchunk + self.chunks_per_seg - 1) // self.chunks_per_seg


def build_nc(cfg: Cfg) -> bass.Bass:
    from concourse import library_config

    B, D, H = cfg.b, cfg.d, cfg.h
    BD = B * D              # 256 elements per table row
    NC = cfg.nchunk
    PLAN = cfg.gather_plan
    NG = len(PLAN)
    CPG = cfg.chunks_per_gather
    NGB = cfg.gat_bufs
    IW1 = 1536 // 16        # idx words per chunk
    # chunk -> (gather idx, offset within gather); gather -> idx-word start
    chunk_gather = {}
    g_iw0 = []
    iw = 0
    for gi, (c0, nch) in enumerate(PLAN):
        g_iw0.append(iw)
        for k in range(nch):
            chunk_gather[c0 + k] = (gi, k)
        iw += nch * IW1
    TOT_IW = iw

    nc = bass.Bass(trn_type="TRN2", num_swdge_queues=4)
    e2c = nc.declare_dram_parameter(
        "e2c", [cfg.nseg, cfg.seg_cap, BD], BF16, isOutput=False)
    gidx = nc.declare_dram_parameter("gidx", [128, TOT_IW], I16, isOutput=False)
    w1blk = nc.declare_dram_parameter("w1blk", [128, 3, 2 * H], BF16, isOutput=False)
    w2q = nc.declare_dram_parameter("w2q", [4 * H, B], BF16, isOutput=False)
    b1q = nc.declare_dram_parameter("b1q", [4 * H, 1], F32, isOutput=False)
    b2q = nc.declare_dram_parameter("b2q", [B, 1], F32, isOutput=False)
    iden = nc.declare_dram_parameter("iden", [128, 128], BF16, isOutput=False)
    o2d = nc.declare_dram_parameter("o2d", [B, cfg.gpad], F32, isOutput=True)

    with ExitStack() as ctx:
        # gathered rows: [g-in-block 128, buf, block (CPG*12), row 256]
        gat = ctx.enter_context(
            nc.sbuf_tensor("gat", [128, NGB, CPG * 12, BD], BF16))
        gidx_sb = ctx.enter_context(nc.sbuf_tensor("gidx_sb", [128, TOT_IW], I16))
        dscr = ctx.enter_context(nc.sbuf_tensor("dscr", [128, 4, BD], BF16))
        w1_sb = ctx.enter_context(nc.sbuf_tensor("w1_sb", [128, 3, 2 * H], BF16))
        w2_sb = ctx.enter_context(nc.sbuf_tensor("w2_sb", [4 * H, B], BF16))
        b1_sb = ctx.enter_context(nc.sbuf_tensor("b1_sb", [4 * H, 1], F32))
        b2_sb = ctx.enter_context(nc.sbuf_tensor("b2_sb", [B, 1], F32))
        id_sb = ctx.enter_context(nc.sbuf_tensor("id_sb", [128, 128], BF16))
        # post-transpose activations: [feat, pair01 512 | pair23 512] x2 bufs
        xsb = [
            ctx.enter_context(nc.sbuf_tensor(f"xsb{sl}", [128, 2, 1024], BF16))
            for sl in range(3)
        ]
        h_sb = ctx.enter_context(nc.sbuf_tensor("h_sb", [4 * H, 2, 512], BF16))
        o_sb = ctx.enter_context(nc.sbuf_tensor("o_sb", [B, 2, 512], F32))
        # PSUM: 3 transpose slabs (bf16, 1 bank per buf) + h/o slab (f32)
        pt = [
            ctx.enter_context(nc.psum_tensor(f"pt{sl}", [128, 2, 1024], BF16))
            for sl in range(3)
        ]
        pho = ctx.enter_context(nc.psum_tensor("pho", [4 * H + B, 2, 512], F32))

        io = ctx.enter_context(nc.semaphore("io"))
        io2 = ctx.enter_context(nc.semaphore("io2"))
        wio = ctx.enter_context(nc.semaphore("wio"))
        gs = [ctx.enter_context(nc.semaphore(f"gs{c}")) for c in range(8)]
        tp_s = ctx.enter_context(nc.semaphore("tp_s"))
        cpv_s = ctx.enter_context(nc.semaphore("cpv_s"))
        cpa_s = ctx.enter_context(nc.semaphore("cpa_s"))
        mm1_s = ctx.enter_context(nc.semaphore("mm1_s"))
        hs_s = ctx.enter_context(nc.semaphore("hs_s"))
        mm2_s = ctx.enter_context(nc.semaphore("mm2_s"))
        os_s = ctx.enter_context(nc.semaphore("os_s"))
        st_p = [ctx.enter_context(nc.semaphore(f"st_p{i}")) for i in range(2)]
        dm = [ctx.enter_context(nc.semaphore(f"dm{i}")) for i in range(4)]
        block = ctx.enter_context(nc.Block())

        HEADW = g_iw0[4] if NG > 4 else TOT_IW

        @block.sync
        def _(sync):
            sync.dma_start(out=gidx_sb[:, 0:HEADW], in_=gidx[:, 0:HEADW]).then_inc(io, 16)
            if HEADW < TOT_IW:
                sync.dma_start(
                    out=gidx_sb[:, HEADW:], in_=gidx[:, HEADW:]).then_inc(io2, 16)
            sync.dma_start(out=w1_sb[:], in_=w1blk[:]).then_inc(wio, 16)
            sync.dma_start(out=w2_sb[:], in_=w2q[:]).then_inc(wio, 16)
            sync.dma_start(out=b1_sb[:], in_=b1q[:]).then_inc(wio, 16)
            sync.dma_start(out=b2_sb[:], in_=b2q[:]).then_inc(wio, 16)
            sync.dma_start(out=id_sb[:], in_=iden[:]).then_inc(wio, 16)
            for q in range(NC):
                sync.wait_ge(os_s, q + 1)
                if q >= 2:
                    sync.wait_ge(st_p[q % 2], 16 * (q // 2))
                sync.dma_start(
                    out=o2d[:, q * 512:(q + 1) * 512], in_=o_sb[:, q % 2, :]
                ).then_inc(st_p[q % 2], 16)
            for par in range(2):
                sync.wait_ge(st_p[par], 16 * ((NC + 1 - par) // 2))

        @block.gpsimd
        def _(gpsimd):
            gpsimd.load_library(library_config.mlp)
            gpsimd.wait_ge(io, 16)
            for g in range(NG):
                c0, nch = PLAN[g]
                if g == 4 and HEADW < TOT_IW:
                    gpsimd.wait_ge(io2, 16)
                assert c0 // cfg.chunks_per_seg == (c0 + nch - 1) // cfg.chunks_per_seg, (
                    f"gather {g} spans segment boundary: {PLAN[g]}")
                if g >= NGB:
                    # gat buffer free when its prior chunks' transposes done
                    pc0, pn = PLAN[g - NGB]
                    gpsimd.wait_ge(tp_s, 24 * (pc0 + pn))
                if g >= 8:
                    # 2 alternating sems per queue: this wait is 2 rounds
                    # back so the sequencer never stalls on it
                    gpsimd.wait_ge(gs[g % 8], 16 * (g // 8))
                gpsimd.dma_gather(
                    out_ap=gat[:, g % NGB, 0:12 * nch, :],
                    in_ap=e2c[c0 // cfg.chunks_per_seg],
                    idxs_ap=gidx_sb[:, g_iw0[g]:g_iw0[g] + nch * IW1],
                    num_idxs=1536 * nch,
                    num_idxs_reg=1536 * nch,
                    elem_size=BD,
                    transpose=False,
                    single_packet=False,
                    queue_num=(g + 1) % 4,
                ).then_inc(gs[g % 8], 16)
            for c in range(8):
                gpsimd.wait_ge(gs[c], 16 * ((NG - c + 7) // 8))

        def pe_transposes(tensor, q):
            # 24 transposes: [128 g, 128 (bpair,d)] -> bf16 PSUM
            bq = q % 2
            g, koff = chunk_gather[q]
            tensor.wait_ge(gs[g % 8], 16 * (g // 8 + 1))
            if q >= 2:
                tensor.wait_ge(cpv_s, 2 * (q - 1))
                tensor.wait_ge(cpa_s, q - 1)
            cbase = koff * 12
            for j in range(4):
                for sl in range(3):
                    for hf in range(2):
                        tensor.matmul(
                            pt[sl][:, bq, hf * 512 + j * 128:hf * 512 + (j + 1) * 128],
                            gat[:, g % NGB, cbase + j * 3 + sl,
                                hf * 128:(hf + 1) * 128],
                            id_sb[:], is_transpose=True,
                        ).then_inc(tp_s, 1)

        def pe_layer1(tensor, q):
            # layer 1: 2 pairs x 3 slots, K=128 block-diagonal W1
            bq = q % 2
            tensor.wait_ge(cpv_s, 2 * (q + 1))
            tensor.wait_ge(cpa_s, q + 1)
            if q >= 2:
                tensor.wait_ge(hs_s, q - 1)
            for p in range(2):
                for sl in range(3):
                    mm = tensor.matmul(
                        pho[p * 2 * H:(p + 1) * 2 * H, bq, :],
                        w1_sb[:, sl, :],
                        xsb[sl][:, bq, p * 512:(p + 1) * 512],
                        start=(sl == 0), stop=(sl == 2),
                    )
                    if sl == 2:
                        mm.then_inc(mm1_s, 1)

        def pe_layer2(tensor, q):
            # layer 2: K=64, 4-batch block-diagonal W2
            bq = q % 2
            tensor.wait_ge(hs_s, q + 1)
            if q >= 2:
                tensor.wait_ge(os_s, q - 1)
            tensor.matmul(
                pho[4 * H:4 * H + B, bq, :], w2_sb[:], h_sb[:, bq, :],
                start=True, stop=True,
            ).then_inc(mm2_s, 1)

        @block.tensor
        def _(tensor):
            tensor.wait_ge(wio, 16 * 5)
            # software pipeline: T[q] | L1[q-1] | L2[q-2] keeps PE fed while
            # DVE/ACT drain PSUM and ACT computes sigmoids
            for q in range(NC):
                pe_transposes(tensor, q)
                if q >= 1:
                    pe_layer1(tensor, q - 1)
                if q >= 2:
                    pe_layer2(tensor, q - 2)
            pe_layer1(tensor, NC - 1)
            pe_layer2(tensor, NC - 2)
            pe_layer2(tensor, NC - 1)
            tensor.wait_ge(mm2_s, NC)

        @block.vector
        def _(vector):
            for q in range(NC):
                bq = q % 2
                vector.wait_ge(tp_s, 24 * (q + 1))
                if q >= 2:
                    vector.wait_ge(mm1_s, 2 * (q - 1))
                vector.tensor_copy(
                    out=xsb[0][:, bq, :], in_=pt[0][:, bq, :]
                ).then_inc(cpv_s, 1)
                vector.tensor_copy(
                    out=xsb[1][:, bq, :], in_=pt[1][:, bq, :]
                ).then_inc(cpv_s, 1)
            vector.wait_ge(cpv_s, 2 * NC)

        def act_copy(scalar, q):
            bq = q % 2
            scalar.wait_ge(tp_s, 24 * (q + 1))
            if q >= 2:
                scalar.wait_ge(mm1_s, 2 * (q - 1))
            scalar.copy(
                out=xsb[2][:, bq, :], in_=pt[2][:, bq, :]
            ).then_inc(cpa_s, 1)

        def act_hsig(scalar, q):
            bq = q % 2
            scalar.wait_ge(mm1_s, 2 * (q + 1))
            scalar.activation(
                h_sb[:, bq, :], pho[0:4 * H, bq, :],
                mybir.ActivationFunctionType.Sigmoid, bias=b1_sb[:],
            ).then_inc(hs_s, 1)

        def act_osig(scalar, q):
            bq = q % 2
            scalar.wait_ge(mm2_s, q + 1)
            if q >= 2:
                scalar.wait_ge(st_p[q % 2], 16 * (q // 2))
            scalar.activation(
                o_sb[:, bq, :], pho[4 * H:4 * H + B, bq, :],
                mybir.ActivationFunctionType.Sigmoid, bias=b2_sb[:],
            ).then_inc(os_s, 1)

        @block.scalar
        def _(scalar):
            # skewed to match the PE pipeline: C[q] | Hs[q-1] | Os[q-2]
            for q in range(NC):
                act_copy(scalar, q)
                if q >= 1:
                    act_hsig(scalar, q - 1)
                if q >= 2:
                    act_osig(scalar, q - 2)
            act_hsig(scalar, NC - 1)
            act_osig(scalar, NC - 2)
            act_osig(scalar, NC - 1)
            scalar.wait_ge(os_s, NC)

    # populate .instr bytes for InstISA subclasses (the library reload);
    # without this the NEFF compiler fails with "ISA wrong length"
    from concourse.library_overlay import lower_extended_insts
    lower_extended_insts(nc)
    return nc


def host_inputs(cfg: Cfg, atoms_embeddings, grounding_indices, W1, b1, W2, b2):
    import ml_dtypes

    B, D, H = cfg.b, cfg.d, cfg.h
    NC, CPS = cfg.nchunk, cfg.chunks_per_seg
    PLAN = cfg.gather_plan
    Fn = grounding_indices.shape[0]
    e2 = np.ascontiguousarray(
        np.transpose(np.asarray(atoms_embeddings, np.float32), (1, 0, 2))
    ).reshape(cfg.n_atoms, B * D).astype(ml_dtypes.bfloat16)
    iden = np.eye(128, dtype=np.float32).astype(ml_dtypes.bfloat16)
    # wrap map: index position i -> (partition i%16 (replicated), word i//16)
    pmod = (np.arange(128) % 16)[:, None]
    maps = []
    for f in range(Fn):
        gi = np.asarray(grounding_indices[f], np.int64)
        gpadded = np.zeros((cfg.gpad, 3), np.int32)
        gpadded[:cfg.g] = gi.astype(np.int32)
        e2cs = np.zeros((cfg.nseg, cfg.seg_cap, B * D), ml_dtypes.bfloat16)
        inv_all = np.zeros((cfg.gpad, 3), np.int16)
        for s in range(cfg.nseg):
            seg = gpadded[s * CPS * 512:(s + 1) * CPS * 512]
            uniq, inv = np.unique(seg, return_inverse=True)
            assert len(uniq) <= cfg.seg_cap
            e2cs[s, :len(uniq)] = e2[uniq]
            inv_all[s * CPS * 512:(s + 1) * CPS * 512] = (
                inv.reshape(seg.shape).astype(np.int16))
        gidx_cols = []
        for c0, nch in PLAN:
            # gather row index i = c*128 + p lands at gat[p, c, :], with
            # c = chunk_in_gather*12 + j*3 + sl and p = g-in-block
            ni = 1536 * nch
            seg_inv = inv_all[c0 * 512:(c0 + nch) * 512]  # [nch*512, 3]
            arr = (seg_inv
                   .reshape(nch, 4, 128, 3)       # [cq, j, p, sl]
                   .transpose(0, 1, 3, 2)         # [cq, j, sl, p]
                   .reshape(ni))
            words = np.arange(ni // 16)[None, :]
            gidx_cols.append(arr[words * 16 + pmod])              # [128, ni/16]
        gidxf = np.concatenate(gidx_cols, axis=1)
        w1f = np.asarray(W1[f], np.float32)      # [192, 16]
        w2f = np.asarray(W2[f], np.float32)      # [16, 1]
        w1b = np.zeros((128, 3, 2 * H), np.float32)
        for sl in range(3):
            w1b[0:D, sl, 0:H] = w1f[sl * D:(sl + 1) * D]
            w1b[D:2 * D, sl, H:2 * H] = w1f[sl * D:(sl + 1) * D]
        w2b = np.zeros((4 * H, B), np.float32)
        for bp in range(B):
            w2b[bp * H:(bp + 1) * H, bp] = w2f[:, 0]
        b1v = np.asarray(b1[f], np.float32)
        maps.append({
            "e2c": e2cs,
            "gidx": gidxf,
            "w1blk": w1b.astype(ml_dtypes.bfloat16),
            "w2q": w2b.astype(ml_dtypes.bfloat16),
            "b1q": np.tile(b1v, B)[:, None].copy(),
            "b2q": np.full((B, 1), np.float32(np.asarray(b2[f]).ravel()[0])),
            "iden": iden,
        })
    return maps


def assemble(cfg: Cfg, results):
    Fn = len(results)
    out = np.zeros((cfg.b, Fn * cfg.g), np.float32)
    for f in range(Fn):
        out[:, f * cfg.g:(f + 1) * cfg.g] = results[f]["o2d"][:, :cfg.g]
    return out


_RUNTIME = {}


def _get_runtime():
    if "nc" not in _RUNTIME:
        cfg = Cfg()
        _RUNTIME["cfg"] = cfg
        _RUNTIME["nc"] = build_nc(cfg)
    return _RUNTIME["cfg"], _RUNTIME["nc"]


def kernel(atoms_embeddings, grounding_indices, W1, b1, W2, b2):
    from concourse.bass_utils import run_bass_kernel_spmd

    cfg, nc = _get_runtime()
    maps = host_inputs(cfg, atoms_embeddings, grounding_indices, W1, b1, W2, b2)
    res = run_bass_kernel_spmd(nc, maps, list(range(len(maps))))
    return assemble(cfg, [res.results[i] for i in range(len(maps))]).astype(np.float32)


# revision 33
# speedup vs baseline: 1.4692x; 1.0388x over previous
"""Trainium2 Bass kernel for nn_CliquesOutputLayer (self-contained).

kernel(**inputs) -> np.ndarray [4, 160000] float32.

Sharding: one formula per NeuronCore (F = 8 = n_cores).

Gather: InstDMAGatherAnt (`dma_gather`, mlp ucode library, non-transposed,
multi-packet). Q7 descriptor generation runs on one core-pair per
queue_num, so gathers round-robin queue_num 0..3 to use all 4 Q7
core-pairs concurrently (num_swdge_queues=4). Each gather fetches
NI=3072 rows = 2 compute chunks, amortizing the ~10us/pair fixed cost.
(The transposed-gather mode would skip the PE transposes below, but
concurrent transposed gathers corrupt each other through the shared
xbar, and serializing them leaves 4x descriptor-gen throughput on the
table -- non-transpose + PE transposes is strictly faster.)

dma_gather needs int16 indices, so the host compacts the atoms table per
10-chunk segment (<= 15360 draws -> unique rows fit a 16384-row table and
local ids fit int16 deterministically). Table rows pack [b0|b1|b2|b3] x 64
bf16 (512B) so one gathered row serves all 4 batches.

Compute per 512-grounding chunk: 24 PE transposes ([128 g, 128 feat] bf16
-> bf16 PSUM, batch-pair-packed), 3 PSUM->SBUF copies (DVE x2, ACT x1),
layer 1 as 6 accumulating bf16 matmuls (batch pairs (0,1)/(2,3) share
K=128 via block-diagonal W1), one [64,512] sigmoid, layer 2 as one K=64
block-diagonal matmul, one [4,512] sigmoid, store. PE/ACT software
pipelines are skewed so no engine round-trips stall PE.
"""
from contextlib import ExitStack
from dataclasses import dataclass
import numpy as np
import concourse.bass as bass
import concourse.mybir as mybir

F32 = mybir.dt.float32
BF16 = mybir.dt.bfloat16
I16 = mybir.dt.int16


@dataclass
class Cfg:
    n_atoms: int = 100000
    g: int = 20000
    b: int = 4
    d: int = 64
    h: int = 16
    chunks_per_gather: int = 2
    gat_bufs: int = 8
    seg_cap: int = 16384

    @property
    def gpad(self):
        blocks = (self.g + 127) // 128
        align = 4 * self.chunks_per_gather
        return (blocks + align - 1) // align * align * 128

    @property
    def nchunk(self):
        return self.gpad // 512

    @property
    def ngather(self):
        return len(self.gather_plan)

    @property
    def gather_plan(self):
        # list of (start_chunk, n_chunks); big gathers first, then 1-chunk
        # gathers for a fine-grained tail. Count kept a multiple of 4 so
        # queue rounds stay balanced.
        nc_ = self.nchunk
        # head smalls halve the opening Pool-engine sync-block; head must be
        # EVEN so big gathers start at even chunks and never span a
        # compaction-segment boundary (segments are 10 chunks)
        head = 2 if nc_ >= 16 else 0
        tail = 6 if nc_ >= 16 else (nc_ if nc_ < 12 else 4)
        big = nc_ - head - tail
        if big % self.chunks_per_gather:
            tail += big % self.chunks_per_gather
            big -= big % self.chunks_per_gather
        plan = []
        c = 0
        for _ in range(head):
            plan.append((c, 1)); c += 1
        for _ in range(big // self.chunks_per_gather):
            plan.append((c, self.chunks_per_gather)); c += self.chunks_per_gather
        for _ in range(tail):
            plan.append((c, 1)); c += 1
        assert c == nc_ and len(plan) % 4 == 0, (c, nc_, plan)
        return plan

    @property
    def chunks_per_seg(self):
        return max(1, self.seg_cap // 1536)

    @property
    def nseg(self):
        return (self.nchunk + self.chunks_per_seg - 1) // self.chunks_per_seg


def build_nc(cfg: Cfg) -> bass.Bass:
    from concourse import library_config

    B, D, H = cfg.b, cfg.d, cfg.h
    BD = B * D              # 256 elements per table row
    NC = cfg.nchunk
    PLAN = cfg.gather_plan
    NG = len(PLAN)
    CPG = cfg.chunks_per_gather
    NGB = cfg.gat_bufs
    IW1 = 1536 // 16        # idx words per chunk
    # chunk -> (gather idx, offset within gather); gather -> idx-word start
    chunk_gather = {}
    g_iw0 = []
    iw = 0
    for gi, (c0, nch) in enumerate(PLAN):
        g_iw0.append(iw)
        for k in range(nch):
            chunk_gather[c0 + k] = (gi, k)
        iw += nch * IW1
    TOT_IW = iw

    nc = bass.Bass(trn_type="TRN2", num_swdge_queues=4)
    e2c = nc.declare_dram_parameter(
        "e2c", [cfg.nseg, cfg.seg_cap, BD], BF16, isOutput=False)
    gidx = nc.declare_dram_parameter("gidx", [128, TOT_IW], I16, isOutput=False)
    w1blk = nc.declare_dram_parameter("w1blk", [128, 3, 2 * H], BF16, isOutput=False)
    w2q = nc.declare_dram_parameter("w2q", [4 * H, B], BF16, isOutput=False)
    b1q = nc.declare_dram_parameter("b1q", [4 * H, 1], F32, isOutput=False)
    b2q = nc.declare_dram_parameter("b2q", [B, 1], F32, isOutput=False)
    iden = nc.declare_dram_parameter("iden", [128, 128], BF16, isOutput=False)
    o2d = nc.declare_dram_parameter("o2d", [B, cfg.gpad], F32, isOutput=True)

    with ExitStack() as ctx:
        # gathered rows: [g-in-block 128, buf, block (CPG*12), row 256]
        gat = ctx.enter_context(
            nc.sbuf_tensor("gat", [128, NGB, CPG * 12, BD], BF16))
        gidx_sb = ctx.enter_context(nc.sbuf_tensor("gidx_sb", [128, TOT_IW], I16))
        dscr = ctx.enter_context(nc.sbuf_tensor("dscr", [128, 4, BD], BF16))
        w1_sb = ctx.enter_context(nc.sbuf_tensor("w1_sb", [128, 3, 2 * H], BF16))
        w2_sb = ctx.enter_context(nc.sbuf_tensor("w2_sb", [4 * H, B], BF16))
        b1_sb = ctx.enter_context(nc.sbuf_tensor("b1_sb", [4 * H, 1], F32))
        b2_sb = ctx.enter_context(nc.sbuf_tensor("b2_sb", [B, 1], F32))
        id_sb = ctx.enter_context(nc.sbuf_tensor("id_sb", [128, 128], BF16))
        # post-transpose activations: [feat, pair01 512 | pair23 512] x2 bufs
        xsb = [
            ctx.enter_context(nc.sbuf_tensor(f"xsb{sl}", [128, 2, 1024], BF16))
            for sl in range(3)
        ]
        h_sb = ctx.enter_context(nc.sbuf_tensor("h_sb", [4 * H, 2, 512], BF16))
        o_sb = ctx.enter_context(nc.sbuf_tensor("o_sb", [B, 2, 512], F32))
        # PSUM: 3 transpose slabs (bf16, 1 bank per buf) + h/o slab (f32)
        pt = [
            ctx.enter_context(nc.psum_tensor(f"pt{sl}", [128, 2, 1024], BF16))
            for sl in range(3)
        ]
        pho = ctx.enter_context(nc.psum_tensor("pho", [4 * H + B, 2, 512], F32))

        io = ctx.enter_context(nc.semaphore("io"))
        wio = ctx.enter_context(nc.semaphore("wio"))
        gs = [ctx.enter_context(nc.semaphore(f"gs{c}")) for c in range(8)]
        tp_s = ctx.enter_context(nc.semaphore("tp_s"))
        cpv_s = ctx.enter_context(nc.semaphore("cpv_s"))
        cpa_s = ctx.enter_context(nc.semaphore("cpa_s"))
        mm1_s = ctx.enter_context(nc.semaphore("mm1_s"))
        hs_s = ctx.enter_context(nc.semaphore("hs_s"))
        mm2_s = ctx.enter_context(nc.semaphore("mm2_s"))
        os_s = ctx.enter_context(nc.semaphore("os_s"))
        st_p = [ctx.enter_context(nc.semaphore(f"st_p{i}")) for i in range(2)]
        dm = [ctx.enter_context(nc.semaphore(f"dm{i}")) for i in range(4)]
        block = ctx.enter_context(nc.Block())

        @block.sync
        def _(sync):
            sync.dma_start(out=gidx_sb[:], in_=gidx[:]).then_inc(io, 16)
            sync.dma_start(out=w1_sb[:], in_=w1blk[:]).then_inc(wio, 16)
            sync.dma_start(out=w2_sb[:], in_=w2q[:]).then_inc(wio, 16)
            sync.dma_start(out=b1_sb[:], in_=b1q[:]).then_inc(wio, 16)
            sync.dma_start(out=b2_sb[:], in_=b2q[:]).then_inc(wio, 16)
            sync.dma_start(out=id_sb[:], in_=iden[:]).then_inc(wio, 16)
            for q in range(NC):
                sync.wait_ge(os_s, q + 1)
                if q >= 2:
                    sync.wait_ge(st_p[q % 2], 16 * (q // 2))
                sync.dma_start(
                    out=o2d[:, q * 512:(q + 1) * 512], in_=o_sb[:, q % 2, :]
                ).then_inc(st_p[q % 2], 16)
            for par in range(2):
                sync.wait_ge(st_p[par], 16 * ((NC + 1 - par) // 2))

        @block.gpsimd
        def _(gpsimd):
            gpsimd.load_library(library_config.mlp)
            gpsimd.wait_ge(io, 16)
            for g in range(NG):
                c0, nch = PLAN[g]
                assert c0 // cfg.chunks_per_seg == (c0 + nch - 1) // cfg.chunks_per_seg, (
                    f"gather {g} spans segment boundary: {PLAN[g]}")
                if g >= NGB:
                    # gat buffer free when its prior chunks' transposes done
                    pc0, pn = PLAN[g - NGB]
                    gpsimd.wait_ge(tp_s, 24 * (pc0 + pn))
                if g >= 8:
                    # 2 alternating sems per queue: this wait is 2 rounds
                    # back so the sequencer never stalls on it
                    gpsimd.wait_ge(gs[g % 8], 16 * (g // 8))
                gpsimd.dma_gather(
                    out_ap=gat[:, g % NGB, 0:12 * nch, :],
                    in_ap=e2c[c0 // cfg.chunks_per_seg],
                    idxs_ap=gidx_sb[:, g_iw0[g]:g_iw0[g] + nch * IW1],
                    num_idxs=1536 * nch,
                    num_idxs_reg=1536 * nch,
                    elem_size=BD,
                    transpose=False,
                    single_packet=False,
                    queue_num=(g + 1) % 4,
                ).then_inc(gs[g % 8], 16)
            for c in range(8):
                gpsimd.wait_ge(gs[c], 16 * ((NG - c + 7) // 8))

        def pe_transposes(tensor, q):
            # 24 transposes: [128 g, 128 (bpair,d)] -> bf16 PSUM
            bq = q % 2
            g, koff = chunk_gather[q]
            tensor.wait_ge(gs[g % 8], 16 * (g // 8 + 1))
            if q >= 2:
                tensor.wait_ge(cpv_s, 2 * (q - 1))
                tensor.wait_ge(cpa_s, q - 1)
            cbase = koff * 12
            for j in range(4):
                for sl in range(3):
                    for hf in range(2):
                        tensor.matmul(
                            pt[sl][:, bq, hf * 512 + j * 128:hf * 512 + (j + 1) * 128],
                            gat[:, g % NGB, cbase + j * 3 + sl,
                                hf * 128:(hf + 1) * 128],
                            id_sb[:], is_transpose=True,
                        ).then_inc(tp_s, 1)

        def pe_layer1(tensor, q):
            # layer 1: 2 pairs x 3 slots, K=128 block-diagonal W1
            bq = q % 2
            tensor.wait_ge(cpv_s, 2 * (q + 1))
            tensor.wait_ge(cpa_s, q + 1)
            if q >= 2:
                tensor.wait_ge(hs_s, q - 1)
            for p in range(2):
                for sl in range(3):
                    mm = tensor.matmul(
                        pho[p * 2 * H:(p + 1) * 2 * H, bq, :],
                        w1_sb[:, sl, :],
                        xsb[sl][:, bq, p * 512:(p + 1) * 512],
                        start=(sl == 0), stop=(sl == 2),
                    )
                    if sl == 2:
                        mm.then_inc(mm1_s, 1)

        def pe_layer2(tensor, q):
            # layer 2: K=64, 4-batch block-diagonal W2
            bq = q % 2
            tensor.wait_ge(hs_s, q + 1)
            if q >= 2:
                tensor.wait_ge(os_s, q - 1)
            tensor.matmul(
                pho[4 * H:4 * H + B, bq, :], w2_sb[:], h_sb[:, bq, :],
                start=True, stop=True,
            ).then_inc(mm2_s, 1)

        @block.tensor
        def _(tensor):
            tensor.wait_ge(wio, 16 * 5)
            # software pipeline: T[q] | L1[q-1] | L2[q-2] keeps PE fed while
            # DVE/ACT drain PSUM and ACT computes sigmoids
            for q in range(NC):
                pe_transposes(tensor, q)
                if q >= 1:
                    pe_layer1(tensor, q - 1)
                if q >= 2:
                    pe_layer2(tensor, q - 2)
            pe_layer1(tensor, NC - 1)
            pe_layer2(tensor, NC - 2)
            pe_layer2(tensor, NC - 1)
            tensor.wait_ge(mm2_s, NC)

        @block.vector
        def _(vector):
            for q in range(NC):
                bq = q % 2
                vector.wait_ge(tp_s, 24 * (q + 1))
                if q >= 2:
                    vector.wait_ge(mm1_s, 2 * (q - 1))
                vector.tensor_copy(
                    out=xsb[0][:, bq, :], in_=pt[0][:, bq, :]
                ).then_inc(cpv_s, 1)
                vector.tensor_copy(
                    out=xsb[1][:, bq, :], in_=pt[1][:, bq, :]
                ).then_inc(cpv_s, 1)
            vector.wait_ge(cpv_s, 2 * NC)

        def act_copy(scalar, q):
            bq = q % 2
            scalar.wait_ge(tp_s, 24 * (q + 1))
            if q >= 2:
                scalar.wait_ge(mm1_s, 2 * (q - 1))
            scalar.copy(
                out=xsb[2][:, bq, :], in_=pt[2][:, bq, :]
            ).then_inc(cpa_s, 1)

        def act_hsig(scalar, q):
            bq = q % 2
            scalar.wait_ge(mm1_s, 2 * (q + 1))
            scalar.activation(
                h_sb[:, bq, :], pho[0:4 * H, bq, :],
                mybir.ActivationFunctionType.Sigmoid, bias=b1_sb[:],
            ).then_inc(hs_s, 1)

        def act_osig(scalar, q):
            bq = q % 2
            scalar.wait_ge(mm2_s, q + 1)
            if q >= 2:
                scalar.wait_ge(st_p[q % 2], 16 * (q // 2))
            scalar.activation(
                o_sb[:, bq, :], pho[4 * H:4 * H + B, bq, :],
                mybir.ActivationFunctionType.Sigmoid, bias=b2_sb[:],
            ).then_inc(os_s, 1)

        @block.scalar
        def _(scalar):
            # skewed to match the PE pipeline: C[q] | Hs[q-1] | Os[q-2]
            for q in range(NC):
                act_copy(scalar, q)
                if q >= 1:
                    act_hsig(scalar, q - 1)
                if q >= 2:
                    act_osig(scalar, q - 2)
            act_hsig(scalar, NC - 1)
            act_osig(scalar, NC - 2)
            act_osig(scalar, NC - 1)
            scalar.wait_ge(os_s, NC)

    # populate .instr bytes for InstISA subclasses (the library reload);
    # without this the NEFF compiler fails with "ISA wrong length"
    from concourse.library_overlay import lower_extended_insts
    lower_extended_insts(nc)
    return nc


def host_inputs(cfg: Cfg, atoms_embeddings, grounding_indices, W1, b1, W2, b2):
    import ml_dtypes

    B, D, H = cfg.b, cfg.d, cfg.h
    NC, CPS = cfg.nchunk, cfg.chunks_per_seg
    PLAN = cfg.gather_plan
    Fn = grounding_indices.shape[0]
    e2 = np.ascontiguousarray(
        np.transpose(np.asarray(atoms_embeddings, np.float32), (1, 0, 2))
    ).reshape(cfg.n_atoms, B * D).astype(ml_dtypes.bfloat16)
    iden = np.eye(128, dtype=np.float32).astype(ml_dtypes.bfloat16)
    # wrap map: index position i -> (partition i%16 (replicated), word i//16)
    pmod = (np.arange(128) % 16)[:, None]
    maps = []
    for f in range(Fn):
        gi = np.asarray(grounding_indices[f], np.int64)
        gpadded = np.zeros((cfg.gpad, 3), np.int32)
        gpadded[:cfg.g] = gi.astype(np.int32)
        e2cs = np.zeros((cfg.nseg, cfg.seg_cap, B * D), ml_dtypes.bfloat16)
        inv_all = np.zeros((cfg.gpad, 3), np.int16)
        for s in range(cfg.nseg):
            seg = gpadded[s * CPS * 512:(s + 1) * CPS * 512]
            uniq, inv = np.unique(seg, return_inverse=True)
            assert len(uniq) <= cfg.seg_cap
            e2cs[s, :len(uniq)] = e2[uniq]
            inv_all[s * CPS * 512:(s + 1) * CPS * 512] = (
                inv.reshape(seg.shape).astype(np.int16))
        gidx_cols = []
        for c0, nch in PLAN:
            # gather row index i = c*128 + p lands at gat[p, c, :], with
            # c = chunk_in_gather*12 + j*3 + sl and p = g-in-block
            ni = 1536 * nch
            seg_inv = inv_all[c0 * 512:(c0 + nch) * 512]  # [nch*512, 3]
            arr = (seg_inv
                   .reshape(nch, 4, 128, 3)       # [cq, j, p, sl]
                   .transpose(0, 1, 3, 2)         # [cq, j, sl, p]
                   .reshape(ni))
            words = np.arange(ni // 16)[None, :]
            gidx_cols.append(arr[words * 16 + pmod])              # [128, ni/16]
        gidxf = np.concatenate(gidx_cols, axis=1)
        w1f = np.asarray(W1[f], np.float32)      # [192, 16]
        w2f = np.asarray(W2[f], np.float32)      # [16, 1]
        w1b = np.zeros((128, 3, 2 * H), np.float32)
        for sl in range(3):
            w1b[0:D, sl, 0:H] = w1f[sl * D:(sl + 1) * D]
            w1b[D:2 * D, sl, H:2 * H] = w1f[sl * D:(sl + 1) * D]
        w2b = np.zeros((4 * H, B), np.float32)
        for bp in range(B):
            w2b[bp * H:(bp + 1) * H, bp] = w2f[:, 0]
        b1v = np.asarray(b1[f], np.float32)
        maps.append({
            "e2c": e2cs,
            "gidx": gidxf,
            "w1blk": w1b.astype(ml_dtypes.bfloat16),
            "w2q": w2b.astype(ml_dtypes.bfloat16),
            "b1q": np.tile(b1v, B)[:, None].copy(),
            "b2q": np.full((B, 1), np.float32(np.asarray(b2[f]).ravel()[0])),
            "iden": iden,
        })
    return maps


def assemble(cfg: Cfg, results):
    Fn = len(results)
    out = np.zeros((cfg.b, Fn * cfg.g), np.float32)
    for f in range(Fn):
        out[:, f * cfg.g:(f + 1) * cfg.g] = results[f]["o2d"][:, :cfg.g]
    return out


_RUNTIME = {}


def _get_runtime():
    if "nc" not in _RUNTIME:
        cfg = Cfg()
        _RUNTIME["cfg"] = cfg
        _RUNTIME["nc"] = build_nc(cfg)
    return _RUNTIME["cfg"], _RUNTIME["nc"]


def kernel(atoms_embeddings, grounding_indices, W1, b1, W2, b2):
    from concourse.bass_utils import run_bass_kernel_spmd

    cfg, nc = _get_runtime()
    maps = host_inputs(cfg, atoms_embeddings, grounding_indices, W1, b1, W2, b2)
    res = run_bass_kernel_spmd(nc, maps, list(range(len(maps))))
    return assemble(cfg, [res.results[i] for i in range(len(maps))]).astype(np.float32)
